# revision 10
# baseline (speedup 1.0000x reference)
"""NT-Xent loss kernel for 8 Trainium2 NeuronCores — v2.

Math (matches the reference):
  Z = concat(z_i, z_j).reshape(8192, 128); r = row-l2-normalize(Z)
  sim = r @ r.T                                  (8192 x 8192)
  row i: S_i = sum_j exp(2*sim[i, j])            (full row, incl. self)
  loss_i = log(S_i - e^2 + exp(2*sim_pair_i)) - 2*sim_pair_i
  loss   = mean_i(loss_i)
  (d_i = exp(2*sim_self) == e^2 up to bf16 normalization noise, whose
   effect on loss is < 1e-5 relative — folded to the constant.)

Sharding: rows split across 8 cores (1024 each); inputs are host-rotated
per core so one SPMD program serves all cores (self diag at local cols
[0,1024), pair diag at [4096,5120)). Host sums the 8x1024 row losses.

Per-core structure:
  Inputs (host-staged, bf16): zr (row-major tiled, for norms),
  zt (transposed, for the normalized matmul operand), ident.
  Prologue (pipelined in 8 sub-chunks of 1024 rows):
    DVE:  sq = zr*zr (bf16), n2 = reduce(sq)        [fast 2-byte modes]
    Pool: u = rsqrt(n2) via Quake seed + 2 Newton   [idle engine]
    DMA:  u -> DRAM (strided transpose) -> utb[128,1024] (bcast read)
    DVE:  znt_chunk = zt_chunk * utb (bf16)         [no xbar transpose!]
  Main loop (8 row-blocks x 8 col-chunks of 1024, PSUM = 8-bank ring of
  4 x [128,1024] f32 slots; 2 bf16 matmuls per chunk):
    exp+row-sum split across three engines:
      A-chunks: ACT exp (scale=2) in-place with fused accum  (exact)
      Q-chunks: ACT exp -> SBUF bf16, Pool sums (no accum)   (exact)
      V-chunks: DVE 1-op Schraudolph exp2 (f32->int16 bf16-bit trick)
                + DVE bf16 reduce                             (~3% elem,
                mean-centered; bias on the final loss < 2e-4)
    pair-sim extracted exactly from PSUM (pre-exp) via ident-mul+accum.
  Epilogue: S = sum of partials, loss = ln(S - e^2 + exp(2 simp)) - 2 simp.
"""

import sys

import numpy as np

sys.path.insert(0, "/opt/trn_rl_repo")

from contextlib import ExitStack  # noqa: E402

import concourse.bass as bass  # noqa: E402
import concourse.tile as tile  # noqa: E402
from concourse import bacc, mybir  # noqa: E402
from concourse.bass_utils import run_bass_kernel_spmd  # noqa: E402

try:
    import ml_dtypes  # noqa: E402

    BF16_NP = ml_dtypes.bfloat16
except ImportError:  # pragma: no cover
    BF16_NP = None

P = 128
N_CORES = 8
NROWS = 8192  # 2N
D = 128
ROWS_PER_CORE = NROWS // N_CORES  # 1024
RB = ROWS_PER_CORE // P  # 8 row blocks per core
SC = 8  # sub-chunks (prologue) == col chunks per row block
SCR = NROWS // SC  # 1024
TPS = SCR // P  # 8 tiles per sub-chunk
MM_N = 512  # one PSUM bank of f32
NSLOT = 4  # PSUM ring slots of 1024 f32 (2 banks each)

# Schraudolph exp2-in-bf16-bits: int16 = round(sim*A + B); bits as bf16
# give exp(2*sim) with ~3% max element error, mean-centered (validated
# on the real input distribution: |rel err| of the loss < 1.3e-4).
A_SCH = 2.0 * 128.0 * 1.4426950408889634  # 2*log2(e)*2^7
B_SCH = 16250.0
E2 = 7.38905609893065  # exp(2): the self-similarity term

F32 = mybir.dt.float32
BF16 = mybir.dt.bfloat16
I16 = mybir.dt.int16
U32 = mybir.dt.uint32
AF = mybir.ActivationFunctionType
OP = mybir.AluOpType
AX = mybir.AxisListType

_CACHE = {}


def _bcast_part(ap: bass.AP, n: int) -> bass.AP:
    """Partition(outer)-broadcast view of a [1, F] DRAM ap -> [n, F]."""
    return bass.AP(
        tensor=ap.tensor, offset=ap.offset, ap=[[0, n], *ap.ap[1:]]
    )


def _broadcast_last(ap: bass.AP, n: int) -> bass.AP:
    return bass.AP(tensor=ap.tensor, offset=ap.offset, ap=[*ap.ap, [0, n]])


def _build_nc():
    nc = bacc.Bacc(
        "TRN2", target_bir_lowering=False, debug=False, num_devices=N_CORES
    )
    zr = nc.dram_tensor("zr", [P, NROWS], BF16, kind="ExternalInput").ap()
    zt = nc.dram_tensor("zt", [P, NROWS], BF16, kind="ExternalInput").ap()
    ident = nc.dram_tensor("ident", [P, P], BF16, kind="ExternalInput").ap()
    out = nc.dram_tensor("loss8", [P, RB], F32, kind="ExternalOutput").ap()

    with tile.TileContext(nc) as tc, ExitStack() as ctx:
        zrpool = ctx.enter_context(tc.tile_pool(name="zrpool", bufs=SC))
        ztpool = ctx.enter_context(tc.tile_pool(name="ztpool", bufs=SC))
        sqpool = ctx.enter_context(tc.tile_pool(name="sqpool", bufs=2))
        small = ctx.enter_context(tc.tile_pool(name="small", bufs=4))
        utbpool = ctx.enter_context(tc.tile_pool(name="utbpool", bufs=3))
        udpool = ctx.enter_context(
            tc.tile_pool(name="udpool", bufs=2, space="DRAM")
        )
        i16pool = ctx.enter_context(tc.tile_pool(name="i16pool", bufs=3))
        exqpool = ctx.enter_context(tc.tile_pool(name="exqpool", bufs=2))
        qdpool = ctx.enter_context(tc.tile_pool(name="qdpool", bufs=2))
        dmpool = ctx.enter_context(tc.tile_pool(name="dmpool", bufs=2))
        singles = ctx.enter_context(tc.tile_pool(name="singles", bufs=1))
        psum = ctx.enter_context(tc.tile_pool(name="psum", bufs=1, space="PSUM"))

        znt = singles.tile([P, NROWS], BF16)  # normalized, transposed
        Ssum = singles.tile([P, RB * SC], F32)  # per (rb, chunk) partials
        simp = singles.tile([P, RB], F32)  # exact pair sims
        sb_ident = singles.tile([P, P], BF16)
        ring = psum.tile([P, NSLOT * 1024], F32)  # all 8 PSUM banks

        nc.vector.memset(Ssum[:], 0.0)

        # ---- input loads ----
        # zt on the scalar queue (done ~6 sub-chunk times in, before the
        # first ACT exp needs the queue); zr + u round-trips on sync,
        # interleaved so utb_c lands just before the normalize-mul needs it.
        zts, zrs = [], []
        for c in range(SC):
            t = ztpool.tile([P, SCR], BF16)
            nc.scalar.dma_start(out=t[:], in_=zt[:, c * SCR : (c + 1) * SCR])
            zts.append(t)
        nc.scalar.dma_start(out=sb_ident[:], in_=ident)
        for c in range(SC):
            t = zrpool.tile([P, TPS, D], BF16)
            zrs.append(t)
        # sync-queue emission: zr0..zr3 first, then (ud_c, utb_c) paired
        # with the remaining zr loads so nothing starves.
        for c in range(4):
            nc.sync.dma_start(out=zrs[c][:], in_=zr[:, c * SCR : (c + 1) * SCR])

        us = [None] * SC  # u bf16 [P, TPS] per sub-chunk
        n2s = [None] * SC

        def norm_stage(c):
            """DVE square + reduce, Pool quake rsqrt for sub-chunk c."""
            zrt = zrs[c]
            sq = sqpool.tile([P, TPS, D], BF16)
            nc.vector.tensor_mul(sq[:], zrt[:], zrt[:])
            n2 = small.tile([P, TPS], F32)
            nc.vector.tensor_reduce(n2[:], sq[:], axis=AX.X, op=OP.add)
            # Quake rsqrt on DVE (the Pool engine's V3 ISA has no generic
            # elementwise ops). Seed 0x5F3759DF - (bits >> 1) built as
            # bits*(-0.5) + magic in the promoted-f32 domain (one op, no
            # shift); the ~1e-5 relative rounding noise is swallowed by the
            # two Newton steps.
            y = small.tile([P, TPS], F32)
            nc.vector.tensor_scalar(
                y[:].bitcast(U32),
                n2[:].bitcast(U32),
                -0.5,
                float(0x5F3759DF),
                OP.mult,
                OP.add,
            )
            u16 = small.tile([P, TPS], BF16)
            for it in range(2):
                t2 = small.tile([P, TPS], F32)
                nc.vector.tensor_mul(t2[:], y[:], y[:])
                nc.vector.scalar_tensor_tensor(
                    out=t2[:], in0=t2[:], scalar=-0.5, in1=n2[:],
                    op0=OP.mult, op1=OP.mult,
                )
                dst = y if it == 0 else u16
                nc.vector.scalar_tensor_tensor(
                    out=dst[:], in0=t2[:], scalar=1.5, in1=y[:],
                    op0=OP.add, op1=OP.mult,
                )
            us[c] = u16
            n2s[c] = n2

        def u_dma_stage(c):
            """u[128,8] -> DRAM (transposed) -> utb[128,1024] bcast."""
            ud = udpool.tile([1, SCR], BF16)
            # ud[0, t*128+p] = u[p, t]
            nc.sync.dma_start(
                out=ud[:].rearrange("a (t p) -> a p t", p=P), in_=us[c][:]
            )
            utb = utbpool.tile([P, SCR], BF16)
            nc.sync.dma_start(out=utb[:], in_=_bcast_part(ud[:], P))
            return utb

        def mul_stage(c, utb):
            nc.vector.tensor_mul(
                znt[:, c * SCR : (c + 1) * SCR], zts[c][:], utb[:]
            )

        # software-pipelined emission (lookahead so in-order engines never
        # head-of-line block): norms run 2 sub-chunks ahead of the muls.
        utbs = [None] * SC
        norm_stage(0)
        norm_stage(1)
        utbs[0] = u_dma_stage(0)
        for c in range(SC):
            if c + 4 < SC:
                nc.sync.dma_start(
                    out=zrs[c + 4][:],
                    in_=zr[:, (c + 4) * SCR : (c + 5) * SCR],
                )
            if c + 2 < SC:
                norm_stage(c + 2)
            if c + 1 < SC:
                utbs[c + 1] = u_dma_stage(c + 1)
            mul_stage(c, utbs[c])

        # ---- main loop ----
        # consumer schedule per row block: chunk index -> consumer, emitted
        # at the LAST chunk of its span (so both fills exist):
        #   'A' = ACT exp+accum over the 2048 span (cc-1, cc)
        #   'V' = DVE Schraudolph over this 1024 chunk
        # rb 0 takes its V chunks last (cc 6,7) so the DVE main-loop ops
        # queue up after the prologue muls without blocking the PSUM ring.
        # One rb goes V-heavy to hit the ACT/DVE load-balance point
        # (a=46, v=18 chunks of 1024).
        sched_rb0 = {1: ("A", 2), 3: ("A", 2), 5: ("A", 2), 6: ("V", 1), 7: ("V", 1)}
        sched = {1: ("A", 2), 3: ("A", 2), 4: ("V", 1), 5: ("V", 1), 7: ("A", 2)}
        sched_vheavy = {1: ("A", 2), 3: ("A", 2), 4: ("V", 1), 5: ("V", 1),
                        6: ("V", 1), 7: ("V", 1)}

        for rb in range(RB):
            plan = sched_rb0 if rb == 0 else (sched_vheavy if rb == 3 else sched)
            for cc in range(SC):
                k = rb * SC + cc
                slot = k % NSLOT
                base = slot * 1024
                for s in range(2):
                    nc.tensor.matmul(
                        ring[:, base + s * MM_N : base + (s + 1) * MM_N],
                        znt[:, rb * P : (rb + 1) * P],
                        znt[:, cc * SCR + s * MM_N : cc * SCR + (s + 1) * MM_N],
                        start=True,
                        stop=True,
                    )
                if cc == 4:
                    # exact pair-sim from PSUM before anything exps it
                    # (pair diag lives at cols [4096,5120) = chunk 4,
                    # in-chunk offset rb*128; slot of cc=4 is 0)
                    dm = dmpool.tile([P, P], F32)
                    nc.vector.scalar_tensor_tensor(
                        out=dm[:],
                        in0=ring[:, rb * P : rb * P + P],
                        scalar=1.0,
                        in1=sb_ident[:],
                        op0=OP.mult,
                        op1=OP.mult,
                        accum_out=simp[:, rb : rb + 1],
                    )
                ent = plan.get(cc)
                if ent is None:
                    continue
                con, width = ent
                col = Ssum[:, k : k + 1]
                sbase = ((k - width + 1) % NSLOT) * 1024
                span = ring[:, sbase : sbase + width * 1024]
                if con == "A":
                    nc.scalar.activation(
                        out=span, in_=span, func=AF.Exp, scale=2.0,
                        accum_out=col,
                    )
                elif con == "Q":
                    ex = exqpool.tile([P, width * 1024], BF16)
                    nc.scalar.activation(
                        out=ex[:], in_=span, func=AF.Exp, scale=2.0
                    )
                    qd = qdpool.tile([P, width * 1024], BF16)
                    nc.gpsimd.tensor_scalar(
                        qd[:], ex[:], 1.0, 0.0, OP.mult, OP.add, accum_out=col
                    )
                else:  # V
                    it16 = i16pool.tile([P, width * 1024], I16)
                    nc.vector.tensor_scalar(
                        it16[:], span, A_SCH, B_SCH, OP.mult, OP.add
                    )
                    nc.vector.tensor_reduce(
                        col, it16[:].bitcast(BF16), axis=AX.X, op=OP.add
                    )

        # ---- epilogue ----
        S8 = singles.tile([P, RB], F32)
        nc.vector.tensor_reduce(
            S8[:], Ssum[:].rearrange("p (r c) -> p r c", c=SC),
            axis=AX.X, op=OP.add,
        )
        p8 = singles.tile([P, RB], F32)
        nc.scalar.activation(out=p8[:], in_=simp[:], func=AF.Exp, scale=2.0)
        # S8 <- S8 - e^2 + p8
        nc.vector.scalar_tensor_tensor(
            out=S8[:], in0=p8[:], scalar=-E2, in1=S8[:], op0=OP.add, op1=OP.add
        )
        lse = singles.tile([P, RB], F32)
        nc.scalar.activation(out=lse[:], in_=S8[:], func=AF.Ln)
        loss8 = singles.tile([P, RB], F32)
        nc.vector.scalar_tensor_tensor(
            out=loss8[:], in0=simp[:], scalar=-2.0, in1=lse[:],
            op0=OP.mult, op1=OP.add,
        )
        nc.sync.dma_start(out=out, in_=loss8[:])

    nc.compile()
    return nc


def get_nc():
    if "nc" not in _CACHE:
        _CACHE["nc"] = _build_nc()
    return _CACHE["nc"]


def make_in_maps(z_i: np.ndarray, z_j: np.ndarray):
    Z = np.concatenate(
        [
            np.asarray(z_i, np.float32).reshape(NROWS // 2, D),
            np.asarray(z_j, np.float32).reshape(NROWS // 2, D),
        ],
        axis=0,
    )
    ident = np.eye(P, dtype=np.float32).astype(BF16_NP)
    in_maps = []
    for k in range(N_CORES):
        zk = np.roll(Z, -k * ROWS_PER_CORE, axis=0)
        zr = np.ascontiguousarray(
            zk.reshape(NROWS // P, P, D).transpose(1, 0, 2)
        ).reshape(P, NROWS).astype(BF16_NP)
        zt = np.ascontiguousarray(zk.T).astype(BF16_NP)
        in_maps.append({"zr": zr, "zt": zt, "ident": ident})
    return in_maps


def run_full(z_i: np.ndarray, z_j: np.ndarray, trace: bool = False):
    nc = get_nc()
    in_maps = make_in_maps(z_i, z_j)
    res = run_bass_kernel_spmd(nc, in_maps, list(range(N_CORES)), trace=trace)
    total = 0.0
    for k in range(N_CORES):
        total += float(np.asarray(res.results[k]["loss8"], np.float64).sum())
    loss = np.float32(total / NROWS)
    return loss, res


def kernel(z_i: np.ndarray, z_j: np.ndarray) -> np.ndarray:
    loss, _ = run_full(z_i, z_j, trace=False)
    return np.asarray(loss, dtype=np.float32)


# revision 22
# speedup vs baseline: 1.1761x; 1.1761x over previous
"""NT-Xent loss kernel for 8 Trainium2 NeuronCores — v2.

Math (matches the reference):
  Z = concat(z_i, z_j).reshape(8192, 128); r = row-l2-normalize(Z)
  sim = r @ r.T                                  (8192 x 8192)
  row i: S_i = sum_j exp(2*sim[i, j])            (full row, incl. self)
  loss_i = log(S_i - e^2 + exp(2*sim_pair_i)) - 2*sim_pair_i
  loss   = mean_i(loss_i)
  (d_i = exp(2*sim_self) == e^2 up to bf16 normalization noise, whose
   effect on loss is < 1e-5 relative — folded to the constant.)

Sharding: rows split across 8 cores (1024 each); inputs are host-rotated
per core so one SPMD program serves all cores (self diag at local cols
[0,1024), pair diag at [4096,5120)). Host sums the 8x1024 row losses.

Per-core structure:
  Inputs (host-staged, bf16): zr (row-major tiled, for norms),
  zt (transposed, for the normalized matmul operand), ident.
  Prologue (pipelined in 8 sub-chunks of 1024 rows):
    DVE:  sq = zr*zr (bf16), n2 = reduce(sq)        [fast 2-byte modes]
    Pool: u = rsqrt(n2) via Quake seed + 2 Newton   [idle engine]
    DMA:  u -> DRAM (strided transpose) -> utb[128,1024] (bcast read)
    DVE:  znt_chunk = zt_chunk * utb (bf16)         [no xbar transpose!]
  Main loop (8 row-blocks x 8 col-chunks of 1024, PSUM = 8-bank ring of
  4 x [128,1024] f32 slots; 2 bf16 matmuls per chunk):
    exp+row-sum split across three engines:
      A-chunks: ACT exp (scale=2) in-place with fused accum  (exact)
      Q-chunks: ACT exp -> SBUF bf16, Pool sums (no accum)   (exact)
      V-chunks: DVE 1-op Schraudolph exp2 (f32->int16 bf16-bit trick)
                + DVE bf16 reduce                             (~3% elem,
                mean-centered; bias on the final loss < 2e-4)
    pair-sim extracted exactly from PSUM (pre-exp) via ident-mul+accum.
  Epilogue: S = sum of partials, loss = ln(S - e^2 + exp(2 simp)) - 2 simp.
"""

import sys

import numpy as np

sys.path.insert(0, "/opt/trn_rl_repo")

from contextlib import ExitStack  # noqa: E402

import concourse.bass as bass  # noqa: E402
import concourse.tile as tile  # noqa: E402
from concourse import bacc, mybir  # noqa: E402
from concourse.bass_utils import run_bass_kernel_spmd  # noqa: E402

try:
    import ml_dtypes  # noqa: E402

    BF16_NP = ml_dtypes.bfloat16
except ImportError:  # pragma: no cover
    BF16_NP = None

P = 128
N_CORES = 8
NROWS = 8192  # 2N
D = 128
ROWS_PER_CORE = NROWS // N_CORES  # 1024
RB = ROWS_PER_CORE // P  # 8 row blocks per core
SC = 8  # sub-chunks (prologue) == col chunks per row block
SCR = NROWS // SC  # 1024
TPS = SCR // P  # 8 tiles per sub-chunk
MM_N = 512  # one PSUM bank of f32
NSLOT = 4  # PSUM ring slots of 1024 f32 (2 banks each)

# Schraudolph exp2-in-bf16-bits: int16 = round(sim*A + B); bits as bf16
# give exp(2*sim) with ~3% max element error, mean-centered (validated
# on the real input distribution: |rel err| of the loss < 1.3e-4).
A_SCH = 2.0 * 128.0 * 1.4426950408889634  # 2*log2(e)*2^7
B_SCH = 16250.0
E2 = 7.38905609893065  # exp(2): the self-similarity term

F32 = mybir.dt.float32
BF16 = mybir.dt.bfloat16
I16 = mybir.dt.int16
U32 = mybir.dt.uint32
AF = mybir.ActivationFunctionType
OP = mybir.AluOpType
AX = mybir.AxisListType

_CACHE = {}


def _bcast_part(ap: bass.AP, n: int) -> bass.AP:
    """Partition(outer)-broadcast view of a [1, F] DRAM ap -> [n, F]."""
    return bass.AP(
        tensor=ap.tensor, offset=ap.offset, ap=[[0, n], *ap.ap[1:]]
    )


def _broadcast_last(ap: bass.AP, n: int) -> bass.AP:
    return bass.AP(tensor=ap.tensor, offset=ap.offset, ap=[*ap.ap, [0, n]])


def _build_nc():
    nc = bacc.Bacc(
        "TRN2", target_bir_lowering=False, debug=False, num_devices=N_CORES
    )
    zr = nc.dram_tensor("zr", [P, NROWS], BF16, kind="ExternalInput").ap()
    zt = nc.dram_tensor("zt", [P, NROWS], BF16, kind="ExternalInput").ap()
    ident = nc.dram_tensor("ident", [P, P], BF16, kind="ExternalInput").ap()
    out = nc.dram_tensor("loss8", [P, RB], F32, kind="ExternalOutput").ap()

    with tile.TileContext(nc) as tc, ExitStack() as ctx:
        zrpool = ctx.enter_context(tc.tile_pool(name="zrpool", bufs=SC))
        ztpool = ctx.enter_context(tc.tile_pool(name="ztpool", bufs=SC))
        sqpool = ctx.enter_context(tc.tile_pool(name="sqpool", bufs=2))
        small = ctx.enter_context(tc.tile_pool(name="small", bufs=4))
        utbpool = ctx.enter_context(tc.tile_pool(name="utbpool", bufs=3))
        udpool = ctx.enter_context(
            tc.tile_pool(name="udpool", bufs=2, space="DRAM")
        )
        utpool = ctx.enter_context(tc.tile_pool(name="utpool", bufs=2))
        i16pool = ctx.enter_context(tc.tile_pool(name="i16pool", bufs=3))
        exqpool = ctx.enter_context(tc.tile_pool(name="exqpool", bufs=2))
        qdpool = ctx.enter_context(tc.tile_pool(name="qdpool", bufs=2))
        dmpool = ctx.enter_context(tc.tile_pool(name="dmpool", bufs=2))
        singles = ctx.enter_context(tc.tile_pool(name="singles", bufs=1))
        psum = ctx.enter_context(tc.tile_pool(name="psum", bufs=1, space="PSUM"))

        znt = singles.tile([P, NROWS], BF16)  # normalized, transposed
        Ssum = singles.tile([P, RB * SC], F32)  # per (rb, chunk) partials
        simp = singles.tile([P, RB], F32)  # exact pair sims
        sb_ident = singles.tile([P, P], BF16)
        ring = psum.tile([P, NSLOT * 1024], F32)  # all 8 PSUM banks

        nc.vector.memset(Ssum[:], 0.0)

        # ---- input loads ----
        # zr loads go on the scalar hwdge queue (prologue-only; the queue is
        # clear again by the time the first ACT exp issues). zt loads, the
        # ident, and the tiny u-transpose DMAs ride the sync queue.
        zts, zrs = [], []
        for c in range(SC):
            zrs.append(zrpool.tile([P, TPS, D], BF16, name="zrt"))
            zts.append(ztpool.tile([P, SCR], BF16, name="ztt"))
        for c in range(SC):
            nc.scalar.dma_start(out=zrs[c][:], in_=zr[:, c * SCR : (c + 1) * SCR])
        nc.sync.dma_start(out=sb_ident[:], in_=ident)
        for c in range(2):
            nc.sync.dma_start(out=zts[c][:], in_=zt[:, c * SCR : (c + 1) * SCR])

        n2all = singles.tile([P, SC * TPS], F32)
        # u16 values live in the first 64 cols of a 128-wide pad so the
        # xbar transpose (which needs free % 128 == 0) can flip them.
        u16all = singles.tile([P, P], BF16)
        uT = singles.tile([P, P], BF16)

        def norm_stage(c):
            """DVE square + reduce for sub-chunk c -> n2all[:, c*8:(c+1)*8]."""
            zrt = zrs[c]
            sq = sqpool.tile([P, TPS, D], BF16)
            nc.vector.tensor_mul(sq[:], zrt[:], zrt[:])
            nc.vector.tensor_reduce(
                n2all[:, c * TPS : (c + 1) * TPS], sq[:], axis=AX.X, op=OP.add
            )

        def quake_stage(cp):
            """Quake rsqrt on DVE for the sub-chunk PAIR (cp, cp+1) at
            [128, 16] grain (amortizes per-op overhead). Seed
            0x5F3759DF - (bits >> 1) built as bits*(-0.5) + magic in the
            promoted-f32 domain; ~1e-5 seed noise is swallowed by the two
            Newton steps."""
            n2 = n2all[:, cp * TPS : (cp + 2) * TPS]
            u16 = u16all[:, cp * TPS : (cp + 2) * TPS]
            y = small.tile([P, 2 * TPS], F32)
            nc.vector.tensor_scalar(
                y[:].bitcast(U32),
                n2.bitcast(U32),
                -0.5,
                float(0x5F3759DF),
                OP.mult,
                OP.add,
            )
            for it in range(2):
                t2 = small.tile([P, 2 * TPS], F32)
                nc.vector.tensor_mul(t2[:], y[:], y[:])
                nc.vector.scalar_tensor_tensor(
                    out=t2[:], in0=t2[:], scalar=-0.5, in1=n2,
                    op0=OP.mult, op1=OP.mult,
                )
                dst = y[:] if it == 0 else u16
                nc.vector.scalar_tensor_tensor(
                    out=dst, in0=t2[:], scalar=1.5, in1=y[:],
                    op0=OP.add, op1=OP.mult,
                )

        def xbar_stage():
            """Transpose the whole u16 pad via the DMA xbar (fast at
            partition-crossing, unlike plain strided DMA which explodes
            into 128 tiny descriptors): uT[f, p] = u16all[p, f]."""
            nc.sync.dma_start(
                out=uT[:].rearrange("a (b c) -> a b c", b=1),
                in_=u16all[:],
                transpose=True,
            )

        def u_stage(c):
            """uT rows [c*8, (c+1)*8) hold u for sub-chunk c's 8 tiles in
            transposed order. Concat them to ut[1,1024] (8 contiguous-row
            descriptors), bounce through DRAM (1-descriptor contiguous
            write), and read back partition-broadcast as utb[128,1024]."""
            ud = udpool.tile([1, SCR], BF16)
            nc.sync.dma_start(out=ud[:], in_=uT[c * TPS : (c + 1) * TPS, :])
            utb = utbpool.tile([P, SCR], BF16)
            nc.sync.dma_start(out=utb[:], in_=_bcast_part(ud[:], P))
            return utb

        def mul_stage(c, utb):
            nc.vector.tensor_mul(
                znt[:, c * SCR : (c + 1) * SCR], zts[c][:], utb[:]
            )

        # software-pipelined emission (lookahead so in-order engines never
        # head-of-line block): norms run 2 sub-chunks ahead of the muls.
        nc.vector.memset(u16all[:], 0.0)
        utbs = [None] * SC
        norm_stage(0)
        norm_stage(1)
        quake_stage(0)
        xbar_stage()
        utbs[0] = u_stage(0)
        for c in range(SC):
            if c + 2 < SC:
                nc.sync.dma_start(
                    out=zts[c + 2][:], in_=zt[:, (c + 2) * SCR : (c + 3) * SCR]
                )
                norm_stage(c + 2)
                if (c + 2) % 2 == 1:
                    quake_stage(c + 1)
                    xbar_stage()
            if c + 1 < SC:
                utbs[c + 1] = u_stage(c + 1)
            mul_stage(c, utbs[c])

        # ---- main loop ----
        # consumer schedule per row block: chunk index -> consumer, emitted
        # at the LAST chunk of its span (so both fills exist):
        #   'A' = ACT exp+accum over the 2048 span (cc-1, cc)
        #   'V' = DVE Schraudolph over this 1024 chunk
        # rb 0-2 are all-ACT: the DVE is busy with the normalization
        # pipeline for the first ~25us, and any main-loop DVE op emitted
        # behind it would block its PSUM ring slot (in-order engine) and
        # stall the PE + ACT. By rb 3 the prologue has drained.
        # Totals: a=48, v=16 chunks of 1024 (ACT/DVE balance point).
        sched_all_a = {1: ("A", 2), 3: ("A", 2), 5: ("A", 2), 7: ("A", 2)}
        sched_v2 = {1: ("A", 2), 3: ("A", 2), 4: ("V", 1), 5: ("V", 1),
                    7: ("A", 2)}
        sched_v4 = {1: ("A", 2), 3: ("A", 2), 4: ("V", 1), 5: ("V", 1),
                    6: ("V", 1), 7: ("V", 1)}

        for rb in range(RB):
            if rb <= 2:
                plan = sched_all_a
            elif rb in (3, 4):
                plan = sched_v2
            else:
                plan = sched_v4
            for cc in range(SC):
                k = rb * SC + cc
                slot = k % NSLOT
                base = slot * 1024
                for s in range(2):
                    nc.tensor.matmul(
                        ring[:, base + s * MM_N : base + (s + 1) * MM_N],
                        znt[:, rb * P : (rb + 1) * P],
                        znt[:, cc * SCR + s * MM_N : cc * SCR + (s + 1) * MM_N],
                        start=True,
                        stop=True,
                    )
                if cc == 4:
                    # exact pair-sim from PSUM before anything exps it
                    # (pair diag lives at cols [4096,5120) = chunk 4,
                    # in-chunk offset rb*128; slot of cc=4 is 0)
                    dm = dmpool.tile([P, P], F32)
                    nc.vector.scalar_tensor_tensor(
                        out=dm[:],
                        in0=ring[:, rb * P : rb * P + P],
                        scalar=1.0,
                        in1=sb_ident[:],
                        op0=OP.mult,
                        op1=OP.mult,
                        accum_out=simp[:, rb : rb + 1],
                    )
                ent = plan.get(cc)
                if ent is None:
                    continue
                con, width = ent
                col = Ssum[:, k : k + 1]
                sbase = ((k - width + 1) % NSLOT) * 1024
                span = ring[:, sbase : sbase + width * 1024]
                if con == "A":
                    nc.scalar.activation(
                        out=span, in_=span, func=AF.Exp, scale=2.0,
                        accum_out=col,
                    )
                elif con == "Q":
                    ex = exqpool.tile([P, width * 1024], BF16)
                    nc.scalar.activation(
                        out=ex[:], in_=span, func=AF.Exp, scale=2.0
                    )
                    qd = qdpool.tile([P, width * 1024], BF16)
                    nc.gpsimd.tensor_scalar(
                        qd[:], ex[:], 1.0, 0.0, OP.mult, OP.add, accum_out=col
                    )
                else:  # V
                    it16 = i16pool.tile([P, width * 1024], I16)
                    nc.vector.tensor_scalar(
                        it16[:], span, A_SCH, B_SCH, OP.mult, OP.add
                    )
                    # row-sum of the bf16-bit exp values via STT-with-accum
                    # ((x*0)+x): 2-byte fast path, unlike TensorReduce
                    # which always runs 1x.
                    vd = qdpool.tile([P, width * 1024], BF16)
                    nc.vector.scalar_tensor_tensor(
                        out=vd[:], in0=it16[:].bitcast(BF16), scalar=0.0,
                        in1=it16[:].bitcast(BF16), op0=OP.mult, op1=OP.add,
                        accum_out=col,
                    )

        # ---- epilogue ----
        S8 = singles.tile([P, RB], F32)
        nc.vector.tensor_reduce(
            S8[:], Ssum[:].rearrange("p (r c) -> p r c", c=SC),
            axis=AX.X, op=OP.add,
        )
        p8 = singles.tile([P, RB], F32)
        nc.scalar.activation(out=p8[:], in_=simp[:], func=AF.Exp, scale=2.0)
        # S8 <- S8 - e^2 + p8
        nc.vector.scalar_tensor_tensor(
            out=S8[:], in0=p8[:], scalar=-E2, in1=S8[:], op0=OP.add, op1=OP.add
        )
        lse = singles.tile([P, RB], F32)
        nc.scalar.activation(out=lse[:], in_=S8[:], func=AF.Ln)
        loss8 = singles.tile([P, RB], F32)
        nc.vector.scalar_tensor_tensor(
            out=loss8[:], in0=simp[:], scalar=-2.0, in1=lse[:],
            op0=OP.mult, op1=OP.add,
        )
        nc.sync.dma_start(out=out, in_=loss8[:])

    nc.compile()
    return nc


def get_nc():
    if "nc" not in _CACHE:
        _CACHE["nc"] = _build_nc()
    return _CACHE["nc"]


def make_in_maps(z_i: np.ndarray, z_j: np.ndarray):
    Z = np.concatenate(
        [
            np.asarray(z_i, np.float32).reshape(NROWS // 2, D),
            np.asarray(z_j, np.float32).reshape(NROWS // 2, D),
        ],
        axis=0,
    )
    ident = np.eye(P, dtype=np.float32).astype(BF16_NP)
    in_maps = []
    for k in range(N_CORES):
        zk = np.roll(Z, -k * ROWS_PER_CORE, axis=0)
        zr = np.ascontiguousarray(
            zk.reshape(NROWS // P, P, D).transpose(1, 0, 2)
        ).reshape(P, NROWS).astype(BF16_NP)
        zt = np.ascontiguousarray(zk.T).astype(BF16_NP)
        in_maps.append({"zr": zr, "zt": zt, "ident": ident})
    return in_maps


def run_full(z_i: np.ndarray, z_j: np.ndarray, trace: bool = False):
    nc = get_nc()
    in_maps = make_in_maps(z_i, z_j)
    res = run_bass_kernel_spmd(nc, in_maps, list(range(N_CORES)), trace=trace)
    total = 0.0
    for k in range(N_CORES):
        total += float(np.asarray(res.results[k]["loss8"], np.float64).sum())
    loss = np.float32(total / NROWS)
    return loss, res


def kernel(z_i: np.ndarray, z_j: np.ndarray) -> np.ndarray:
    loss, _ = run_full(z_i, z_j, trace=False)
    return np.asarray(loss, dtype=np.float32)


# revision 25
# speedup vs baseline: 1.4823x; 1.2603x over previous
"""NT-Xent loss kernel for 8 Trainium2 NeuronCores — v2.

Math (matches the reference):
  Z = concat(z_i, z_j).reshape(8192, 128); r = row-l2-normalize(Z)
  sim = r @ r.T                                  (8192 x 8192)
  row i: S_i = sum_j exp(2*sim[i, j])            (full row, incl. self)
  loss_i = log(S_i - e^2 + exp(2*sim_pair_i)) - 2*sim_pair_i
  loss   = mean_i(loss_i)
  (d_i = exp(2*sim_self) == e^2 up to bf16 normalization noise, whose
   effect on loss is < 1e-5 relative — folded to the constant.)

Sharding: rows split across 8 cores (1024 each); inputs are host-rotated
per core so one SPMD program serves all cores (self diag at local cols
[0,1024), pair diag at [4096,5120)). Host sums the 8x1024 row losses.

Per-core structure:
  Inputs (host-staged, bf16): zr (row-major tiled, for norms),
  zt (transposed, for the normalized matmul operand), ident.
  Prologue (pipelined in 8 sub-chunks of 1024 rows):
    DVE:  sq = zr*zr (bf16), n2 = reduce(sq)        [fast 2-byte modes]
    Pool: u = rsqrt(n2) via Quake seed + 2 Newton   [idle engine]
    DMA:  u -> DRAM (strided transpose) -> utb[128,1024] (bcast read)
    DVE:  znt_chunk = zt_chunk * utb (bf16)         [no xbar transpose!]
  Main loop (8 row-blocks x 8 col-chunks of 1024, PSUM = 8-bank ring of
  4 x [128,1024] f32 slots; 2 bf16 matmuls per chunk):
    exp+row-sum split across three engines:
      A-chunks: ACT exp (scale=2) in-place with fused accum  (exact)
      Q-chunks: ACT exp -> SBUF bf16, Pool sums (no accum)   (exact)
      V-chunks: DVE 1-op Schraudolph exp2 (f32->int16 bf16-bit trick)
                + DVE bf16 reduce                             (~3% elem,
                mean-centered; bias on the final loss < 2e-4)
    pair-sim extracted exactly from PSUM (pre-exp) via ident-mul+accum.
  Epilogue: S = sum of partials, loss = ln(S - e^2 + exp(2 simp)) - 2 simp.
"""

import sys

import numpy as np

sys.path.insert(0, "/opt/trn_rl_repo")

from contextlib import ExitStack  # noqa: E402

import concourse.bass as bass  # noqa: E402
import concourse.tile as tile  # noqa: E402
from concourse import bacc, mybir  # noqa: E402
from concourse.bass_utils import run_bass_kernel_spmd  # noqa: E402

try:
    import ml_dtypes  # noqa: E402

    BF16_NP = ml_dtypes.bfloat16
except ImportError:  # pragma: no cover
    BF16_NP = None

P = 128
N_CORES = 8
NROWS = 8192  # 2N
D = 128
ROWS_PER_CORE = NROWS // N_CORES  # 1024
RB = ROWS_PER_CORE // P  # 8 row blocks per core
SC = 8  # sub-chunks (prologue) == col chunks per row block
SCR = NROWS // SC  # 1024
TPS = SCR // P  # 8 tiles per sub-chunk
MM_N = 512  # one PSUM bank of f32
NSLOT = 4  # PSUM ring slots of 1024 f32 (2 banks each)

# Schraudolph exp2-in-bf16-bits: int16 = round(sim*A + B); bits as bf16
# give exp(2*sim) with ~3% max element error, mean-centered (validated
# on the real input distribution: |rel err| of the loss < 1.3e-4).
A_SCH = 2.0 * 128.0 * 1.4426950408889634  # 2*log2(e)*2^7
B_SCH = 16250.0
E2 = 7.38905609893065  # exp(2): the self-similarity term

F32 = mybir.dt.float32
BF16 = mybir.dt.bfloat16
I16 = mybir.dt.int16
U32 = mybir.dt.uint32
AF = mybir.ActivationFunctionType
OP = mybir.AluOpType
AX = mybir.AxisListType

_CACHE = {}


def _bcast_part(ap: bass.AP, n: int) -> bass.AP:
    """Partition(outer)-broadcast view of a [1, F] DRAM ap -> [n, F]."""
    return bass.AP(
        tensor=ap.tensor, offset=ap.offset, ap=[[0, n], *ap.ap[1:]]
    )


def _broadcast_last(ap: bass.AP, n: int) -> bass.AP:
    return bass.AP(tensor=ap.tensor, offset=ap.offset, ap=[*ap.ap, [0, n]])


def _build_nc():
    nc = bacc.Bacc(
        "TRN2", target_bir_lowering=False, debug=False, num_devices=N_CORES
    )
    zr = nc.dram_tensor("zr", [P, NROWS], BF16, kind="ExternalInput").ap()
    zt = nc.dram_tensor("zt", [P, NROWS], BF16, kind="ExternalInput").ap()
    ident = nc.dram_tensor("ident", [P, P], BF16, kind="ExternalInput").ap()
    out = nc.dram_tensor("loss8", [P, RB], F32, kind="ExternalOutput").ap()

    with tile.TileContext(nc) as tc, ExitStack() as ctx:
        zrpool = ctx.enter_context(tc.tile_pool(name="zrpool", bufs=SC))
        ztpool = ctx.enter_context(tc.tile_pool(name="ztpool", bufs=SC))
        sqpool = ctx.enter_context(tc.tile_pool(name="sqpool", bufs=2))
        small = ctx.enter_context(tc.tile_pool(name="small", bufs=4))
        utbpool = ctx.enter_context(tc.tile_pool(name="utbpool", bufs=3))
        udpool = ctx.enter_context(
            tc.tile_pool(name="udpool", bufs=2, space="DRAM")
        )
        utpool = ctx.enter_context(tc.tile_pool(name="utpool", bufs=2))
        i16pool = ctx.enter_context(tc.tile_pool(name="i16pool", bufs=3))
        exqpool = ctx.enter_context(tc.tile_pool(name="exqpool", bufs=2))
        qdpool = ctx.enter_context(tc.tile_pool(name="qdpool", bufs=2))
        dmpool = ctx.enter_context(tc.tile_pool(name="dmpool", bufs=2))
        singles = ctx.enter_context(tc.tile_pool(name="singles", bufs=1))
        psum = ctx.enter_context(tc.tile_pool(name="psum", bufs=1, space="PSUM"))

        znt = singles.tile([P, NROWS], BF16)  # normalized, transposed
        Ssum = singles.tile([P, RB * SC], F32)  # per (rb, chunk) partials
        simp = singles.tile([P, RB], F32)  # exact pair sims
        sb_ident = singles.tile([P, P], BF16)
        ring = psum.tile([P, NSLOT * 1024], F32)  # all 8 PSUM banks

        nc.vector.memset(Ssum[:], 0.0)

        # ---- input loads ----
        # zr loads go on the scalar hwdge queue (prologue-only; the queue is
        # clear again by the time the first ACT exp issues). zt loads, the
        # ident, and the tiny u-transpose DMAs ride the sync queue.
        zts, zrs = [], []
        for c in range(SC):
            zrs.append(zrpool.tile([P, TPS, D], BF16, name="zrt"))
            zts.append(ztpool.tile([P, SCR], BF16, name="ztt"))
        for c in range(SC):
            nc.scalar.dma_start(out=zrs[c][:], in_=zr[:, c * SCR : (c + 1) * SCR])
        nc.sync.dma_start(out=sb_ident[:], in_=ident)
        # all zt loads up front on sync: nothing they depend on, and the
        # u-chain DMAs behind them would otherwise head-of-line block them
        for c in range(SC):
            nc.sync.dma_start(out=zts[c][:], in_=zt[:, c * SCR : (c + 1) * SCR])

        n2all = singles.tile([P, SC * TPS], F32)
        # u16 values live in the first 64 cols of a 128-wide pad so the
        # xbar transpose (which needs free % 128 == 0) can flip them.
        u16all = singles.tile([P, P], BF16)
        uT = singles.tile([P, P], BF16)

        def norm_stage(c):
            """DVE square + reduce for sub-chunk c -> n2all[:, c*8:(c+1)*8]."""
            zrt = zrs[c]
            sq = sqpool.tile([P, TPS, D], BF16)
            nc.vector.tensor_mul(sq[:], zrt[:], zrt[:])
            nc.vector.tensor_reduce(
                n2all[:, c * TPS : (c + 1) * TPS], sq[:], axis=AX.X, op=OP.add
            )

        def quake_stage(cp):
            """Quake rsqrt on DVE for the sub-chunk PAIR (cp, cp+1) at
            [128, 16] grain (amortizes per-op overhead). Seed
            0x5F3759DF - (bits >> 1) built as bits*(-0.5) + magic in the
            promoted-f32 domain; ~1e-5 seed noise is swallowed by the two
            Newton steps."""
            n2 = n2all[:, cp * TPS : (cp + 2) * TPS]
            u16 = u16all[:, cp * TPS : (cp + 2) * TPS]
            y = small.tile([P, 2 * TPS], F32)
            nc.vector.tensor_scalar(
                y[:].bitcast(U32),
                n2.bitcast(U32),
                -0.5,
                float(0x5F3759DF),
                OP.mult,
                OP.add,
            )
            for it in range(2):
                t2 = small.tile([P, 2 * TPS], F32)
                nc.vector.tensor_mul(t2[:], y[:], y[:])
                nc.vector.scalar_tensor_tensor(
                    out=t2[:], in0=t2[:], scalar=-0.5, in1=n2,
                    op0=OP.mult, op1=OP.mult,
                )
                dst = y[:] if it == 0 else u16
                nc.vector.scalar_tensor_tensor(
                    out=dst, in0=t2[:], scalar=1.5, in1=y[:],
                    op0=OP.add, op1=OP.mult,
                )

        def xbar_stage():
            """Transpose the whole u16 pad via the DMA xbar (fast at
            partition-crossing, unlike plain strided DMA which explodes
            into 128 tiny descriptors): uT[f, p] = u16all[p, f]."""
            nc.sync.dma_start(
                out=uT[:].rearrange("a (b c) -> a b c", b=1),
                in_=u16all[:],
                transpose=True,
            )

        def u_stage(c):
            """uT rows [c*8, (c+1)*8) hold u for sub-chunk c's 8 tiles in
            transposed order. Concat them to ut[1,1024] (8 contiguous-row
            descriptors), bounce through DRAM (1-descriptor contiguous
            write), and read back partition-broadcast as utb[128,1024]."""
            ud = udpool.tile([1, SCR], BF16)
            nc.sync.dma_start(out=ud[:], in_=uT[c * TPS : (c + 1) * TPS, :])
            utb = utbpool.tile([P, SCR], BF16)
            nc.sync.dma_start(out=utb[:], in_=_bcast_part(ud[:], P))
            return utb

        def mul_stage(c, utb):
            nc.vector.tensor_mul(
                znt[:, c * SCR : (c + 1) * SCR], zts[c][:], utb[:]
            )

        # software-pipelined emission (lookahead so in-order engines never
        # head-of-line block): norms run 2 sub-chunks ahead of the muls.
        nc.vector.memset(u16all[:], 0.0)
        utbs = [None] * SC
        norm_stage(0)
        norm_stage(1)
        quake_stage(0)
        xbar_stage()
        utbs[0] = u_stage(0)
        for c in range(SC):
            if c + 2 < SC:
                norm_stage(c + 2)
                if (c + 2) % 2 == 1:
                    quake_stage(c + 1)
                    xbar_stage()
            if c + 1 < SC:
                utbs[c + 1] = u_stage(c + 1)
            mul_stage(c, utbs[c])

        # ---- main loop ----
        # Every row block needs every znt sub-chunk, but column chunk cc
        # only needs znt sub-chunk cc. So phase 1 sweeps cc 0..1 across all
        # row blocks (consumable as soon as mul0/mul1 land, while the
        # normalization pipeline still runs), then phase 2 goes row-block-
        # outer over cc 2..7 with efficient 2048-wide ACT spans.
        #   'A' = ACT exp+accum (in place, fused row-sum)
        #   'V' = DVE Schraudolph (int16 bf16-bit exp2) + STT-accum row-sum
        # V work is scheduled late (rb >= 3 in phase 2) so it queues behind
        # the prologue on the in-order DVE without blocking the PSUM ring.
        kctr = [0]

        def fill(rb, cc):
            slot = kctr[0] % NSLOT
            kctr[0] += 1
            base = slot * 1024
            for s in range(2):
                nc.tensor.matmul(
                    ring[:, base + s * MM_N : base + (s + 1) * MM_N],
                    znt[:, rb * P : (rb + 1) * P],
                    znt[:, cc * SCR + s * MM_N : cc * SCR + (s + 1) * MM_N],
                    start=True,
                    stop=True,
                )
            if cc == 4:
                # exact pair-sim from PSUM before anything exps it (pair
                # diag lives at cols [4096,5120) = chunk 4, in-chunk offset
                # rb*128)
                dm = dmpool.tile([P, P], F32)
                nc.vector.scalar_tensor_tensor(
                    out=dm[:],
                    in0=ring[:, base + rb * P : base + rb * P + P],
                    scalar=1.0,
                    in1=sb_ident[:],
                    op0=OP.mult,
                    op1=OP.mult,
                    accum_out=simp[:, rb : rb + 1],
                )
            return base

        def consume(con, base, width, col):
            span = ring[:, base : base + width * 1024]
            if con == "A":
                nc.scalar.activation(
                    out=span, in_=span, func=AF.Exp, scale=2.0, accum_out=col
                )
            else:  # V
                it16 = i16pool.tile([P, width * 1024], I16)
                nc.vector.tensor_scalar(
                    it16[:], span, A_SCH, B_SCH, OP.mult, OP.add
                )
                # row-sum of the bf16-bit exp values via STT-with-accum
                # ((x*0)+x): 2-byte fast path, unlike TensorReduce which
                # always runs 1x.
                vd = qdpool.tile([P, width * 1024], BF16)
                nc.vector.scalar_tensor_tensor(
                    out=vd[:], in0=it16[:].bitcast(BF16), scalar=0.0,
                    in1=it16[:].bitcast(BF16), op0=OP.mult, op1=OP.add,
                    accum_out=col,
                )

        # phase 1: cc 0..1, all row blocks, 1024-wide ACT chunks
        for cc in range(2):
            for rb in range(RB):
                base = fill(rb, cc)
                consume("A", base, 1, Ssum[:, rb * SC + cc : rb * SC + cc + 1])
        # phase 2: row-block outer, cc 2..7; spans of 2 chunks where both
        # chunks belong to the same rb. rb 6-7 go V-heavy for load balance.
        for rb in range(RB):
            plan = (
                [("A", 2, 3), ("V", 1, 4), ("V", 1, 5), ("A", 2, 7)]
                if rb < 6
                else [("A", 2, 3), ("V", 2, 5), ("V", 2, 7)]
            )
            pend = {}
            for cc in range(2, SC):
                base = fill(rb, cc)
                pend[cc] = base
                for con, width, end_cc in plan:
                    if end_cc == cc:
                        sbase = pend[cc - width + 1]
                        col = Ssum[:, rb * SC + cc : rb * SC + cc + 1]
                        consume(con, sbase, width, col)

        # ---- epilogue ----
        S8 = singles.tile([P, RB], F32)
        nc.vector.tensor_reduce(
            S8[:], Ssum[:].rearrange("p (r c) -> p r c", c=SC),
            axis=AX.X, op=OP.add,
        )
        p8 = singles.tile([P, RB], F32)
        nc.scalar.activation(out=p8[:], in_=simp[:], func=AF.Exp, scale=2.0)
        # S8 <- S8 - e^2 + p8
        nc.vector.scalar_tensor_tensor(
            out=S8[:], in0=p8[:], scalar=-E2, in1=S8[:], op0=OP.add, op1=OP.add
        )
        lse = singles.tile([P, RB], F32)
        nc.scalar.activation(out=lse[:], in_=S8[:], func=AF.Ln)
        loss8 = singles.tile([P, RB], F32)
        nc.vector.scalar_tensor_tensor(
            out=loss8[:], in0=simp[:], scalar=-2.0, in1=lse[:],
            op0=OP.mult, op1=OP.add,
        )
        nc.sync.dma_start(out=out, in_=loss8[:])

    nc.compile()
    return nc


def get_nc():
    if "nc" not in _CACHE:
        _CACHE["nc"] = _build_nc()
    return _CACHE["nc"]


def make_in_maps(z_i: np.ndarray, z_j: np.ndarray):
    Z = np.concatenate(
        [
            np.asarray(z_i, np.float32).reshape(NROWS // 2, D),
            np.asarray(z_j, np.float32).reshape(NROWS // 2, D),
        ],
        axis=0,
    )
    ident = np.eye(P, dtype=np.float32).astype(BF16_NP)
    in_maps = []
    for k in range(N_CORES):
        zk = np.roll(Z, -k * ROWS_PER_CORE, axis=0)
        zr = np.ascontiguousarray(
            zk.reshape(NROWS // P, P, D).transpose(1, 0, 2)
        ).reshape(P, NROWS).astype(BF16_NP)
        zt = np.ascontiguousarray(zk.T).astype(BF16_NP)
        in_maps.append({"zr": zr, "zt": zt, "ident": ident})
    return in_maps


def run_full(z_i: np.ndarray, z_j: np.ndarray, trace: bool = False):
    nc = get_nc()
    in_maps = make_in_maps(z_i, z_j)
    res = run_bass_kernel_spmd(nc, in_maps, list(range(N_CORES)), trace=trace)
    total = 0.0
    for k in range(N_CORES):
        total += float(np.asarray(res.results[k]["loss8"], np.float64).sum())
    loss = np.float32(total / NROWS)
    return loss, res


def kernel(z_i: np.ndarray, z_j: np.ndarray) -> np.ndarray:
    loss, _ = run_full(z_i, z_j, trace=False)
    return np.asarray(loss, dtype=np.float32)


# revision 33
# speedup vs baseline: 1.5723x; 1.0607x over previous
"""NT-Xent loss kernel for 8 Trainium2 NeuronCores — v2.

Math (matches the reference):
  Z = concat(z_i, z_j).reshape(8192, 128); r = row-l2-normalize(Z)
  sim = r @ r.T                                  (8192 x 8192)
  row i: S_i = sum_j exp(2*sim[i, j])            (full row, incl. self)
  loss_i = log(S_i - e^2 + exp(2*sim_pair_i)) - 2*sim_pair_i
  loss   = mean_i(loss_i)
  (d_i = exp(2*sim_self) == e^2 up to bf16 normalization noise, whose
   effect on loss is < 1e-5 relative — folded to the constant.)

Sharding: rows split across 8 cores (1024 each); inputs are host-rotated
per core so one SPMD program serves all cores (self diag at local cols
[0,1024), pair diag at [4096,5120)). Host sums the 8x1024 row losses.

Per-core structure:
  Inputs (host-staged, bf16): zr (row-major tiled, for norms),
  zt (transposed, for the normalized matmul operand), ident.
  Prologue (pipelined in 8 sub-chunks of 1024 rows):
    DVE:  sq = zr*zr (bf16), n2 = reduce(sq)        [fast 2-byte modes]
    Pool: u = rsqrt(n2) via Quake seed + 2 Newton   [idle engine]
    DMA:  u -> DRAM (strided transpose) -> utb[128,1024] (bcast read)
    DVE:  znt_chunk = zt_chunk * utb (bf16)         [no xbar transpose!]
  Main loop (8 row-blocks x 8 col-chunks of 1024, PSUM = 8-bank ring of
  4 x [128,1024] f32 slots; 2 bf16 matmuls per chunk):
    exp+row-sum split across three engines:
      A-chunks: ACT exp (scale=2) in-place with fused accum  (exact)
      Q-chunks: ACT exp -> SBUF bf16, Pool sums (no accum)   (exact)
      V-chunks: DVE 1-op Schraudolph exp2 (f32->int16 bf16-bit trick)
                + DVE bf16 reduce                             (~3% elem,
                mean-centered; bias on the final loss < 2e-4)
    pair-sim extracted exactly from PSUM (pre-exp) via ident-mul+accum.
  Epilogue: S = sum of partials, loss = ln(S - e^2 + exp(2 simp)) - 2 simp.
"""

import sys

import numpy as np

sys.path.insert(0, "/opt/trn_rl_repo")

from contextlib import ExitStack  # noqa: E402

import concourse.bass as bass  # noqa: E402
import concourse.tile as tile  # noqa: E402
from concourse import bacc, mybir  # noqa: E402
from concourse.bass_utils import run_bass_kernel_spmd  # noqa: E402

try:
    import ml_dtypes  # noqa: E402

    BF16_NP = ml_dtypes.bfloat16
except ImportError:  # pragma: no cover
    BF16_NP = None

P = 128
N_CORES = 8
NROWS = 8192  # 2N
D = 128
ROWS_PER_CORE = NROWS // N_CORES  # 1024
RB = ROWS_PER_CORE // P  # 8 row blocks per core
SC = 8  # sub-chunks (prologue) == col chunks per row block
SCR = NROWS // SC  # 1024
TPS = SCR // P  # 8 tiles per sub-chunk
MM_N = 512  # one PSUM bank of f32
NSLOT = 4  # PSUM ring slots of 1024 f32 (2 banks each)

# Schraudolph exp2-in-bf16-bits: int16 = round(sim*A + B); bits as bf16
# give exp(2*sim) with ~3% max element error, mean-centered (validated
# on the real input distribution: |rel err| of the loss < 1.3e-4).
A_SCH = 2.0 * 128.0 * 1.4426950408889634  # 2*log2(e)*2^7
B_SCH = 16250.0
E2 = 7.38905609893065  # exp(2): the self-similarity term

F32 = mybir.dt.float32
BF16 = mybir.dt.bfloat16
FP8 = mybir.dt.float8e4
I16 = mybir.dt.int16
U32 = mybir.dt.uint32
AF = mybir.ActivationFunctionType
OP = mybir.AluOpType
AX = mybir.AxisListType

_CACHE = {}


def _bcast_part(ap: bass.AP, n: int) -> bass.AP:
    """Partition(outer)-broadcast view of a [1, F] DRAM ap -> [n, F]."""
    return bass.AP(
        tensor=ap.tensor, offset=ap.offset, ap=[[0, n], *ap.ap[1:]]
    )


def _broadcast_last(ap: bass.AP, n: int) -> bass.AP:
    return bass.AP(tensor=ap.tensor, offset=ap.offset, ap=[*ap.ap, [0, n]])


def _build_nc():
    nc = bacc.Bacc(
        "TRN2", target_bir_lowering=False, debug=False, num_devices=N_CORES
    )
    zr = nc.dram_tensor("zr", [P, NROWS], BF16, kind="ExternalInput").ap()
    zt = nc.dram_tensor("zt", [P, NROWS], BF16, kind="ExternalInput").ap()
    ident = nc.dram_tensor("ident", [P, P], BF16, kind="ExternalInput").ap()
    out = nc.dram_tensor("loss8", [P, RB], F32, kind="ExternalOutput").ap()

    with tile.TileContext(nc) as tc, ExitStack() as ctx:
        zrpool = ctx.enter_context(tc.tile_pool(name="zrpool", bufs=SC))
        ztpool = ctx.enter_context(tc.tile_pool(name="ztpool", bufs=SC))
        sqpool = ctx.enter_context(tc.tile_pool(name="sqpool", bufs=2))
        small = ctx.enter_context(tc.tile_pool(name="small", bufs=4))
        utbpool = ctx.enter_context(tc.tile_pool(name="utbpool", bufs=3))
        udpool = ctx.enter_context(
            tc.tile_pool(name="udpool", bufs=2, space="DRAM")
        )
        utpool = ctx.enter_context(tc.tile_pool(name="utpool", bufs=2))
        i16pool = ctx.enter_context(tc.tile_pool(name="i16pool", bufs=14))
        exqpool = ctx.enter_context(tc.tile_pool(name="exqpool", bufs=2))
        qdpool = ctx.enter_context(tc.tile_pool(name="qdpool", bufs=2))
        dmpool = ctx.enter_context(tc.tile_pool(name="dmpool", bufs=2))
        singles = ctx.enter_context(tc.tile_pool(name="singles", bufs=1))
        psum = ctx.enter_context(tc.tile_pool(name="psum", bufs=1, space="PSUM"))

        # normalized, transposed representation in fp8e4, laid out for
        # DoubleRow double-pumped matmuls: k-tile 0 = data, k-tile 1 = zeros
        # (contraction is only 128 deep; the zero tile rides along so the
        # PE processes the chunk at 0.5 cycles/row).
        znt8 = singles.tile([P, 2, NROWS], FP8)
        Ssum = singles.tile([P, RB * SC], F32)  # per (rb, chunk) partials
        simp = singles.tile([P, RB], F32)  # exact pair sims
        sb_ident = singles.tile([P, P], BF16)
        ring = psum.tile([P, NSLOT * 1024], F32)  # all 8 PSUM banks

        nc.vector.memset(Ssum[:], 0.0)
        # zero k-tile for the DoubleRow matmuls, once, on the idle Pool
        nc.gpsimd.memset(znt8[:, 1, :], 0.0)

        # ---- input loads ----
        # zr loads go on the scalar hwdge queue (prologue-only; the queue is
        # clear again by the time the first ACT exp issues). zt loads, the
        # ident, and the tiny u-transpose DMAs ride the sync queue.
        zts, zrs = [], []
        for c in range(SC):
            zrs.append(zrpool.tile([P, TPS, D], BF16, name="zrt"))
            zts.append(ztpool.tile([P, SCR], BF16, name="ztt"))
        for c in range(SC):
            nc.scalar.dma_start(out=zrs[c][:], in_=zr[:, c * SCR : (c + 1) * SCR])
        nc.sync.dma_start(out=sb_ident[:], in_=ident)
        # all zt loads up front on sync: nothing they depend on, and the
        # u-chain DMAs behind them would otherwise head-of-line block them
        for c in range(SC):
            nc.sync.dma_start(out=zts[c][:], in_=zt[:, c * SCR : (c + 1) * SCR])

        n2all = singles.tile([P, SC * TPS], F32)
        # u16 values live in the first 64 cols of a 128-wide pad so the
        # xbar transpose (which needs free % 128 == 0) can flip them.
        u16all = singles.tile([P, P], BF16)
        uT = singles.tile([P, P], BF16)

        def norm_stage(c):
            """DVE square + reduce for sub-chunk c -> n2all[:, c*8:(c+1)*8]."""
            zrt = zrs[c]
            sq = sqpool.tile([P, TPS, D], BF16)
            nc.vector.tensor_mul(sq[:], zrt[:], zrt[:])
            nc.vector.tensor_reduce(
                n2all[:, c * TPS : (c + 1) * TPS], sq[:], axis=AX.X, op=OP.add
            )

        def quake_stage(cp):
            """Quake rsqrt on DVE for the sub-chunk PAIR (cp, cp+1) at
            [128, 16] grain (amortizes per-op overhead). Seed
            0x5F3759DF - (bits >> 1) built as bits*(-0.5) + magic in the
            promoted-f32 domain; ~1e-5 seed noise is swallowed by the two
            Newton steps."""
            n2 = n2all[:, cp * TPS : (cp + 2) * TPS]
            u16 = u16all[:, cp * TPS : (cp + 2) * TPS]
            y = small.tile([P, 2 * TPS], F32)
            nc.vector.tensor_scalar(
                y[:].bitcast(U32),
                n2.bitcast(U32),
                -0.5,
                float(0x5F3759DF),
                OP.mult,
                OP.add,
            )
            # one Newton step (seed err ~3.4% -> ~0.2%; u is bf16 anyway
            # and the residual is random across rows, washing out of the
            # mean loss). Fewer serial DVE hops = shorter critical path.
            t2 = small.tile([P, 2 * TPS], F32)
            nc.vector.tensor_mul(t2[:], y[:], y[:])
            nc.vector.scalar_tensor_tensor(
                out=t2[:], in0=t2[:], scalar=-0.5, in1=n2,
                op0=OP.mult, op1=OP.mult,
            )
            nc.vector.scalar_tensor_tensor(
                out=u16, in0=t2[:], scalar=1.5, in1=y[:],
                op0=OP.add, op1=OP.mult,
            )

        def xbar_stage():
            """Transpose the whole u16 pad via the DMA xbar (fast at
            partition-crossing, unlike plain strided DMA which explodes
            into 128 tiny descriptors): uT[f, p] = u16all[p, f]."""
            nc.sync.dma_start(
                out=uT[:].rearrange("a (b c) -> a b c", b=1),
                in_=u16all[:],
                transpose=True,
            )

        def u_stage(c):
            """uT rows [c*8, (c+1)*8) hold u for sub-chunk c's 8 tiles in
            transposed order. Concat them to ut[1,1024] (8 contiguous-row
            descriptors), bounce through DRAM (1-descriptor contiguous
            write), and read back partition-broadcast as utb[128,1024]."""
            ud = udpool.tile([1, SCR], BF16)
            nc.sync.dma_start(out=ud[:], in_=uT[c * TPS : (c + 1) * TPS, :])
            utb = utbpool.tile([P, SCR], BF16)
            nc.sync.dma_start(out=utb[:], in_=_bcast_part(ud[:], P))
            return utb

        def mul_stage(c, utb):
            nc.vector.tensor_mul(
                znt8[:, 0, c * SCR : (c + 1) * SCR], zts[c][:], utb[:]
            )

        # software-pipelined emission (lookahead so in-order engines never
        # head-of-line block): norms run 2 sub-chunks ahead of the muls.
        nc.vector.memset(u16all[:], 0.0)
        utbs = [None] * SC
        norm_stage(0)
        norm_stage(1)
        quake_stage(0)
        xbar_stage()
        utbs[0] = u_stage(0)
        for c in range(SC):
            if c + 2 < SC:
                norm_stage(c + 2)
                if (c + 2) % 2 == 1:
                    quake_stage(c + 1)
                    xbar_stage()
            if c + 1 < SC:
                utbs[c + 1] = u_stage(c + 1)
            mul_stage(c, utbs[c])

        # ---- main loop ----
        # Every row block needs every znt sub-chunk, but column chunk cc
        # only needs znt sub-chunk cc. So phase 1 sweeps cc 0..1 across all
        # row blocks (consumable as soon as mul0/mul1 land, while the
        # normalization pipeline still runs), then phase 2 goes row-block-
        # outer over cc 2..7 with efficient 2048-wide ACT spans.
        #   'A' = ACT exp+accum (in place, fused row-sum)
        #   'V' = DVE Schraudolph (int16 bf16-bit exp2) + STT-accum row-sum
        # V work is scheduled late (rb >= 3 in phase 2) so it queues behind
        # the prologue on the in-order DVE without blocking the PSUM ring.
        kctr = [0]

        def fill(rb, cc):
            slot = kctr[0] % NSLOT
            kctr[0] += 1
            base = slot * 1024
            for s in range(2):
                nc.tensor.matmul(
                    ring[:, base + s * MM_N : base + (s + 1) * MM_N],
                    znt8[:, :, rb * P : (rb + 1) * P],
                    znt8[
                        :, :, cc * SCR + s * MM_N : cc * SCR + (s + 1) * MM_N
                    ],
                    start=True,
                    stop=True,
                    perf_mode=mybir.MatmulPerfMode.DoubleRow,
                )
            if cc == 4:
                # exact pair-sim from PSUM before anything exps it (pair
                # diag lives at cols [4096,5120) = chunk 4, in-chunk offset
                # rb*128)
                dm = dmpool.tile([P, P], F32)
                nc.vector.scalar_tensor_tensor(
                    out=dm[:],
                    in0=ring[:, base + rb * P : base + rb * P + P],
                    scalar=1.0,
                    in1=sb_ident[:],
                    op0=OP.mult,
                    op1=OP.mult,
                    accum_out=simp[:, rb : rb + 1],
                )
            return base

        deferred = []  # (i16 tile, width, Ssum col) -- row-sums run at the
        # end, hidden under the ACT tail, so they never block the PSUM ring

        def consume(con, base, width, col):
            span = ring[:, base : base + width * 1024]
            if con == "A":
                nc.scalar.activation(
                    out=span, in_=span, func=AF.Exp, scale=2.0, accum_out=col
                )
            else:  # V
                it16 = i16pool.tile([P, width * 1024], I16)
                nc.vector.tensor_scalar(
                    it16[:], span, A_SCH, B_SCH, OP.mult, OP.add
                )
                deferred.append((it16, width, col))

        # phase 1: cc 0..1 as a 2048 ACT span per row block (only needs
        # znt sub-chunks 0-1, so it runs while the prologue still streams)
        for rb in range(RB):
            b0 = fill(rb, 0)
            fill(rb, 1)
            consume("A", b0, 2, Ssum[:, rb * SC : rb * SC + 1])
        # phase 2: row-block outer, cc 2..7; spans pair chunks of the SAME
        # row block. rb7 is all-ACT to balance the deferred DVE row-sums.
        for rb in range(RB):
            plan = (
                [("A", 2, 3), ("V", 1, 4), ("V", 1, 5), ("A", 2, 7)]
                if rb < 7
                else [("A", 2, 3), ("A", 2, 5), ("A", 2, 7)]
            )
            pend = {}
            for cc in range(2, SC):
                base = fill(rb, cc)
                pend[cc] = base
                for con, width, end_cc in plan:
                    if end_cc == cc:
                        sbase = pend[cc - width + 1]
                        col = Ssum[:, rb * SC + cc : rb * SC + cc + 1]
                        consume(con, sbase, width, col)

        # deferred V row-sums: STT-with-accum ((x*0)+x) over the bf16-bit
        # exp values -- 2-byte fast path, unlike TensorReduce (always 1x)
        for it16, width, col in deferred:
            vd = qdpool.tile([P, width * 1024], BF16)
            nc.vector.scalar_tensor_tensor(
                out=vd[:], in0=it16[:].bitcast(BF16), scalar=0.0,
                in1=it16[:].bitcast(BF16), op0=OP.mult, op1=OP.add,
                accum_out=col,
            )

        # ---- epilogue ----
        S8 = singles.tile([P, RB], F32)
        nc.vector.tensor_reduce(
            S8[:], Ssum[:].rearrange("p (r c) -> p r c", c=SC),
            axis=AX.X, op=OP.add,
        )
        p8 = singles.tile([P, RB], F32)
        nc.scalar.activation(out=p8[:], in_=simp[:], func=AF.Exp, scale=2.0)
        # S8 <- S8 - e^2 + p8
        nc.vector.scalar_tensor_tensor(
            out=S8[:], in0=p8[:], scalar=-E2, in1=S8[:], op0=OP.add, op1=OP.add
        )
        lse = singles.tile([P, RB], F32)
        nc.scalar.activation(out=lse[:], in_=S8[:], func=AF.Ln)
        loss8 = singles.tile([P, RB], F32)
        nc.vector.scalar_tensor_tensor(
            out=loss8[:], in0=simp[:], scalar=-2.0, in1=lse[:],
            op0=OP.mult, op1=OP.add,
        )
        nc.sync.dma_start(out=out, in_=loss8[:])

    nc.compile()
    return nc


def get_nc():
    if "nc" not in _CACHE:
        _CACHE["nc"] = _build_nc()
    return _CACHE["nc"]


def make_in_maps(z_i: np.ndarray, z_j: np.ndarray):
    Z = np.concatenate(
        [
            np.asarray(z_i, np.float32).reshape(NROWS // 2, D),
            np.asarray(z_j, np.float32).reshape(NROWS // 2, D),
        ],
        axis=0,
    )
    ident = np.eye(P, dtype=np.float32).astype(BF16_NP)
    in_maps = []
    for k in range(N_CORES):
        zk = np.roll(Z, -k * ROWS_PER_CORE, axis=0)
        zr = np.ascontiguousarray(
            zk.reshape(NROWS // P, P, D).transpose(1, 0, 2)
        ).reshape(P, NROWS).astype(BF16_NP)
        zt = np.ascontiguousarray(zk.T).astype(BF16_NP)
        in_maps.append({"zr": zr, "zt": zt, "ident": ident})
    return in_maps


def run_full(z_i: np.ndarray, z_j: np.ndarray, trace: bool = False):
    nc = get_nc()
    in_maps = make_in_maps(z_i, z_j)
    res = run_bass_kernel_spmd(nc, in_maps, list(range(N_CORES)), trace=trace)
    total = 0.0
    for k in range(N_CORES):
        total += float(np.asarray(res.results[k]["loss8"], np.float64).sum())
    loss = np.float32(total / NROWS)
    return loss, res


def kernel(z_i: np.ndarray, z_j: np.ndarray) -> np.ndarray:
    loss, _ = run_full(z_i, z_j, trace=False)
    return np.asarray(loss, dtype=np.float32)


# revision 39
# speedup vs baseline: 1.6570x; 1.0539x over previous
"""NT-Xent loss kernel for 8 Trainium2 NeuronCores — v2.

Math (matches the reference):
  Z = concat(z_i, z_j).reshape(8192, 128); r = row-l2-normalize(Z)
  sim = r @ r.T                                  (8192 x 8192)
  row i: S_i = sum_j exp(2*sim[i, j])            (full row, incl. self)
  loss_i = log(S_i - e^2 + exp(2*sim_pair_i)) - 2*sim_pair_i
  loss   = mean_i(loss_i)
  (d_i = exp(2*sim_self) == e^2 up to bf16 normalization noise, whose
   effect on loss is < 1e-5 relative — folded to the constant.)

Sharding: rows split across 8 cores (1024 each); inputs are host-rotated
per core so one SPMD program serves all cores (self diag at local cols
[0,1024), pair diag at [4096,5120)). Host sums the 8x1024 row losses.

Per-core structure:
  Inputs (host-staged, bf16): zr (row-major tiled, for norms),
  zt (transposed, for the normalized matmul operand), ident.
  Prologue (pipelined in 8 sub-chunks of 1024 rows):
    DVE:  sq = zr*zr (bf16), n2 = reduce(sq)        [fast 2-byte modes]
    Pool: u = rsqrt(n2) via Quake seed + 2 Newton   [idle engine]
    DMA:  u -> DRAM (strided transpose) -> utb[128,1024] (bcast read)
    DVE:  znt_chunk = zt_chunk * utb (bf16)         [no xbar transpose!]
  Main loop (8 row-blocks x 8 col-chunks of 1024, PSUM = 8-bank ring of
  4 x [128,1024] f32 slots; 2 bf16 matmuls per chunk):
    exp+row-sum split across three engines:
      A-chunks: ACT exp (scale=2) in-place with fused accum  (exact)
      Q-chunks: ACT exp -> SBUF bf16, Pool sums (no accum)   (exact)
      V-chunks: DVE 1-op Schraudolph exp2 (f32->int16 bf16-bit trick)
                + DVE bf16 reduce                             (~3% elem,
                mean-centered; bias on the final loss < 2e-4)
    pair-sim extracted exactly from PSUM (pre-exp) via ident-mul+accum.
  Epilogue: S = sum of partials, loss = ln(S - e^2 + exp(2 simp)) - 2 simp.
"""

import sys

import numpy as np

sys.path.insert(0, "/opt/trn_rl_repo")

from contextlib import ExitStack  # noqa: E402

import concourse.bass as bass  # noqa: E402
import concourse.tile as tile  # noqa: E402
from concourse import bacc, mybir  # noqa: E402
from concourse.bass_utils import run_bass_kernel_spmd  # noqa: E402

try:
    import ml_dtypes  # noqa: E402

    BF16_NP = ml_dtypes.bfloat16
except ImportError:  # pragma: no cover
    BF16_NP = None

P = 128
N_CORES = 8
NROWS = 8192  # 2N
D = 128
ROWS_PER_CORE = NROWS // N_CORES  # 1024
RB = ROWS_PER_CORE // P  # 8 row blocks per core
SC = 8  # sub-chunks (prologue) == col chunks per row block
SCR = NROWS // SC  # 1024
TPS = SCR // P  # 8 tiles per sub-chunk
MM_N = 512  # one PSUM bank of f32
NSLOT = 4  # PSUM ring slots of 1024 f32 (2 banks each)

# Schraudolph exp2-in-bf16-bits: int16 = round(sim*A + B); bits as bf16
# give exp(2*sim) with ~3% max element error, mean-centered (validated
# on the real input distribution: |rel err| of the loss < 1.3e-4).
A_SCH = 2.0 * 128.0 * 1.4426950408889634  # 2*log2(e)*2^7
B_SCH = 16250.0
E2 = 7.38905609893065  # exp(2): the self-similarity term

F32 = mybir.dt.float32
BF16 = mybir.dt.bfloat16
FP8 = mybir.dt.float8e4
I16 = mybir.dt.int16
U32 = mybir.dt.uint32
AF = mybir.ActivationFunctionType
OP = mybir.AluOpType
AX = mybir.AxisListType

_CACHE = {}


def _bcast_part(ap: bass.AP, n: int) -> bass.AP:
    """Partition(outer)-broadcast view of a [1, F] DRAM ap -> [n, F]."""
    return bass.AP(
        tensor=ap.tensor, offset=ap.offset, ap=[[0, n], *ap.ap[1:]]
    )


def _broadcast_last(ap: bass.AP, n: int) -> bass.AP:
    return bass.AP(tensor=ap.tensor, offset=ap.offset, ap=[*ap.ap, [0, n]])


def _build_nc():
    nc = bacc.Bacc(
        "TRN2", target_bir_lowering=False, debug=False, num_devices=N_CORES
    )
    zr = nc.dram_tensor("zr", [P, NROWS], BF16, kind="ExternalInput").ap()
    zt = nc.dram_tensor("zt", [P, NROWS], BF16, kind="ExternalInput").ap()
    ident = nc.dram_tensor("ident", [P, P], BF16, kind="ExternalInput").ap()
    out = nc.dram_tensor("loss8", [P, RB], F32, kind="ExternalOutput").ap()

    with tile.TileContext(nc) as tc, ExitStack() as ctx:
        zrpool = ctx.enter_context(tc.tile_pool(name="zrpool", bufs=SC))
        ztpool = ctx.enter_context(tc.tile_pool(name="ztpool", bufs=SC))
        sqpool = ctx.enter_context(tc.tile_pool(name="sqpool", bufs=2))
        small = ctx.enter_context(tc.tile_pool(name="small", bufs=4))
        utbpool = ctx.enter_context(tc.tile_pool(name="utbpool", bufs=3))
        udpool = ctx.enter_context(
            tc.tile_pool(name="udpool", bufs=2, space="DRAM")
        )
        utpool = ctx.enter_context(tc.tile_pool(name="utpool", bufs=2))
        i16pool = ctx.enter_context(tc.tile_pool(name="i16pool", bufs=14))
        exqpool = ctx.enter_context(tc.tile_pool(name="exqpool", bufs=2))
        qdpool = ctx.enter_context(tc.tile_pool(name="qdpool", bufs=2))
        dmpool = ctx.enter_context(tc.tile_pool(name="dmpool", bufs=2))
        singles = ctx.enter_context(tc.tile_pool(name="singles", bufs=1))
        psum = ctx.enter_context(tc.tile_pool(name="psum", bufs=1, space="PSUM"))

        znt = singles.tile([P, NROWS], BF16)  # normalized, transposed
        Ssum = singles.tile([P, RB * SC], F32)  # per (rb, chunk) partials
        simp = singles.tile([P, RB], F32)  # exact pair sims
        sb_ident = singles.tile([P, P], BF16)
        ring = psum.tile([P, NSLOT * 1024], F32)  # all 8 PSUM banks

        nc.vector.memset(Ssum[:], 0.0)

        # ---- input loads ----
        # zr loads go on the scalar hwdge queue (prologue-only; the queue is
        # clear again by the time the first ACT exp issues). zt loads, the
        # ident, and the tiny u-transpose DMAs ride the sync queue.
        zts, zrs = [], []
        for c in range(SC):
            zrs.append(zrpool.tile([P, TPS, D], BF16, name="zrt"))
            zts.append(ztpool.tile([P, SCR], BF16, name="ztt"))
        for c in range(SC):
            nc.scalar.dma_start(out=zrs[c][:], in_=zr[:, c * SCR : (c + 1) * SCR])
        nc.sync.dma_start(out=sb_ident[:], in_=ident)
        # all zt loads up front on sync: nothing they depend on, and the
        # u-chain DMAs behind them would otherwise head-of-line block them
        for c in range(SC):
            nc.sync.dma_start(out=zts[c][:], in_=zt[:, c * SCR : (c + 1) * SCR])

        n2all = singles.tile([P, SC * TPS], F32)
        # u16 values live in the first 64 cols of a 128-wide pad so the
        # xbar transpose (which needs free % 128 == 0) can flip them.
        u16all = singles.tile([P, P], BF16)
        uT = singles.tile([P, P], BF16)

        def norm_stage(c):
            """DVE square + reduce for sub-chunk c -> n2all[:, c*8:(c+1)*8]."""
            zrt = zrs[c]
            sq = sqpool.tile([P, TPS, D], BF16)
            nc.vector.tensor_mul(sq[:], zrt[:], zrt[:])
            nc.vector.tensor_reduce(
                n2all[:, c * TPS : (c + 1) * TPS], sq[:], axis=AX.X, op=OP.add
            )

        def quake_stage(cp):
            """Quake rsqrt on DVE for the sub-chunk PAIR (cp, cp+1) at
            [128, 16] grain (amortizes per-op overhead). Seed
            0x5F3759DF - (bits >> 1) built as bits*(-0.5) + magic in the
            promoted-f32 domain; ~1e-5 seed noise is swallowed by the two
            Newton steps."""
            n2 = n2all[:, cp * TPS : (cp + 2) * TPS]
            u16 = u16all[:, cp * TPS : (cp + 2) * TPS]
            y = small.tile([P, 2 * TPS], F32)
            nc.vector.tensor_scalar(
                y[:].bitcast(U32),
                n2.bitcast(U32),
                -0.5,
                float(0x5F3759DF),
                OP.mult,
                OP.add,
            )
            # one Newton step (seed err ~3.4% -> ~0.2%; u is bf16 anyway
            # and the residual is random across rows, washing out of the
            # mean loss). Fewer serial DVE hops = shorter critical path.
            t2 = small.tile([P, 2 * TPS], F32)
            nc.vector.tensor_mul(t2[:], y[:], y[:])
            nc.vector.scalar_tensor_tensor(
                out=t2[:], in0=t2[:], scalar=-0.5, in1=n2,
                op0=OP.mult, op1=OP.mult,
            )
            nc.vector.scalar_tensor_tensor(
                out=u16, in0=t2[:], scalar=1.5, in1=y[:],
                op0=OP.add, op1=OP.mult,
            )

        def xbar_stage():
            """Transpose the whole u16 pad via the DMA xbar (fast at
            partition-crossing, unlike plain strided DMA which explodes
            into 128 tiny descriptors): uT[f, p] = u16all[p, f]."""
            nc.sync.dma_start(
                out=uT[:].rearrange("a (b c) -> a b c", b=1),
                in_=u16all[:],
                transpose=True,
            )

        def u_stage(c):
            """uT rows [c*8, (c+1)*8) hold u for sub-chunk c's 8 tiles in
            transposed order. Concat them to ut[1,1024] (8 contiguous-row
            descriptors), bounce through DRAM (1-descriptor contiguous
            write), and read back partition-broadcast as utb[128,1024]."""
            ud = udpool.tile([1, SCR], BF16)
            nc.sync.dma_start(out=ud[:], in_=uT[c * TPS : (c + 1) * TPS, :])
            utb = utbpool.tile([P, SCR], BF16)
            nc.sync.dma_start(out=utb[:], in_=_bcast_part(ud[:], P))
            return utb

        def mul_stage(c, utb):
            nc.vector.tensor_mul(
                znt[:, c * SCR : (c + 1) * SCR], zts[c][:], utb[:]
            )

        # software-pipelined emission (lookahead so in-order engines never
        # head-of-line block): norms run 2 sub-chunks ahead of the muls.
        nc.vector.memset(u16all[:], 0.0)
        utbs = [None] * SC
        norm_stage(0)
        norm_stage(1)
        quake_stage(0)
        xbar_stage()
        utbs[0] = u_stage(0)
        utbs[1] = u_stage(1)
        norm_stage(2)  # fills the u-DMA latency gap on the DVE
        mul_stage(0, utbs[0])
        mul_stage(1, utbs[1])

        def finish_prologue():
            for c in range(2, SC):
                if c + 1 < SC:
                    norm_stage(c + 1)
                    if (c + 1) % 2 == 1:
                        quake_stage(c)
                        xbar_stage()
                utbs[c] = u_stage(c)
                mul_stage(c, utbs[c])

        # ---- main loop ----
        # Every row block needs every znt sub-chunk, but column chunk cc
        # only needs znt sub-chunk cc. So phase 1 sweeps cc 0..1 across all
        # row blocks (consumable as soon as mul0/mul1 land, while the
        # normalization pipeline still runs), then phase 2 goes row-block-
        # outer over cc 2..7 with efficient 2048-wide ACT spans.
        #   'A' = ACT exp+accum (in place, fused row-sum)
        #   'V' = DVE Schraudolph (int16 bf16-bit exp2) + STT-accum row-sum
        # V work is scheduled late (rb >= 3 in phase 2) so it queues behind
        # the prologue on the in-order DVE without blocking the PSUM ring.
        kctr = [0]

        def fill(rb, cc):
            slot = kctr[0] % NSLOT
            kctr[0] += 1
            base = slot * 1024
            for s in range(2):
                nc.tensor.matmul(
                    ring[:, base + s * MM_N : base + (s + 1) * MM_N],
                    znt[:, rb * P : (rb + 1) * P],
                    znt[:, cc * SCR + s * MM_N : cc * SCR + (s + 1) * MM_N],
                    start=True,
                    stop=True,
                )
            if cc == 4:
                # exact pair-sim from PSUM before anything exps it (pair
                # diag lives at cols [4096,5120) = chunk 4, in-chunk offset
                # rb*128)
                dm = dmpool.tile([P, P], F32)
                nc.vector.scalar_tensor_tensor(
                    out=dm[:],
                    in0=ring[:, base + rb * P : base + rb * P + P],
                    scalar=1.0,
                    in1=sb_ident[:],
                    op0=OP.mult,
                    op1=OP.mult,
                    accum_out=simp[:, rb : rb + 1],
                )
            return base

        deferred = []  # (i16 tile, width, Ssum col) -- row-sums run at the
        # end, hidden under the ACT tail, so they never block the PSUM ring

        def consume(con, base, width, col):
            span = ring[:, base : base + width * 1024]
            if con == "A":
                nc.scalar.activation(
                    out=span, in_=span, func=AF.Exp, scale=2.0, accum_out=col
                )
            else:  # V
                it16 = i16pool.tile([P, width * 1024], I16)
                nc.vector.tensor_scalar(
                    it16[:], span, A_SCH, B_SCH, OP.mult, OP.add
                )
                deferred.append((it16, width, col))

        # phase 1: cc 0..1 as a 2048 ACT span per row block (only needs
        # znt sub-chunks 0-1, so it runs while the prologue still streams;
        # emitted BEFORE the rest of the prologue so the Tile scheduler
        # cannot starve the mul0/mul1 chain with later norm work)
        for rb in range(RB):
            b0 = fill(rb, 0)
            fill(rb, 1)
            consume("A", b0, 2, Ssum[:, rb * SC : rb * SC + 1])
        finish_prologue()
        # phase 2: row-block outer, cc 2..7; spans pair chunks of the SAME
        # row block. rb 0-1 all-ACT (the DVE is still draining the
        # prologue; a V op emitted behind it would block its PSUM slot).
        for rb in range(RB):
            plan = (
                [("A", 2, 3), ("V", 1, 4), ("V", 1, 5), ("A", 2, 7)]
                if rb >= 2
                else [("A", 2, 3), ("A", 2, 5), ("A", 2, 7)]
            )
            pend = {}
            for cc in range(2, SC):
                base = fill(rb, cc)
                pend[cc] = base
                for con, width, end_cc in plan:
                    if end_cc == cc:
                        sbase = pend[cc - width + 1]
                        col = Ssum[:, rb * SC + cc : rb * SC + cc + 1]
                        consume(con, sbase, width, col)

        # deferred V row-sums: STT-with-accum ((x*0)+x) over the bf16-bit
        # exp values -- 2-byte fast path, unlike TensorReduce (always 1x)
        for it16, width, col in deferred:
            vd = qdpool.tile([P, width * 1024], BF16)
            nc.vector.scalar_tensor_tensor(
                out=vd[:], in0=it16[:].bitcast(BF16), scalar=0.0,
                in1=it16[:].bitcast(BF16), op0=OP.mult, op1=OP.add,
                accum_out=col,
            )

        # ---- epilogue ----
        S8 = singles.tile([P, RB], F32)
        nc.vector.tensor_reduce(
            S8[:], Ssum[:].rearrange("p (r c) -> p r c", c=SC),
            axis=AX.X, op=OP.add,
        )
        p8 = singles.tile([P, RB], F32)
        nc.scalar.activation(out=p8[:], in_=simp[:], func=AF.Exp, scale=2.0)
        # S8 <- S8 - e^2 + p8
        nc.vector.scalar_tensor_tensor(
            out=S8[:], in0=p8[:], scalar=-E2, in1=S8[:], op0=OP.add, op1=OP.add
        )
        lse = singles.tile([P, RB], F32)
        nc.scalar.activation(out=lse[:], in_=S8[:], func=AF.Ln)
        loss8 = singles.tile([P, RB], F32)
        nc.vector.scalar_tensor_tensor(
            out=loss8[:], in0=simp[:], scalar=-2.0, in1=lse[:],
            op0=OP.mult, op1=OP.add,
        )
        nc.sync.dma_start(out=out, in_=loss8[:])

    nc.compile()
    return nc


def get_nc():
    if "nc" not in _CACHE:
        _CACHE["nc"] = _build_nc()
    return _CACHE["nc"]


def make_in_maps(z_i: np.ndarray, z_j: np.ndarray):
    Z = np.concatenate(
        [
            np.asarray(z_i, np.float32).reshape(NROWS // 2, D),
            np.asarray(z_j, np.float32).reshape(NROWS // 2, D),
        ],
        axis=0,
    )
    ident = np.eye(P, dtype=np.float32).astype(BF16_NP)
    in_maps = []
    for k in range(N_CORES):
        zk = np.roll(Z, -k * ROWS_PER_CORE, axis=0)
        zr = np.ascontiguousarray(
            zk.reshape(NROWS // P, P, D).transpose(1, 0, 2)
        ).reshape(P, NROWS).astype(BF16_NP)
        zt = np.ascontiguousarray(zk.T).astype(BF16_NP)
        in_maps.append({"zr": zr, "zt": zt, "ident": ident})
    return in_maps


def run_full(z_i: np.ndarray, z_j: np.ndarray, trace: bool = False):
    nc = get_nc()
    in_maps = make_in_maps(z_i, z_j)
    res = run_bass_kernel_spmd(nc, in_maps, list(range(N_CORES)), trace=trace)
    total = 0.0
    for k in range(N_CORES):
        total += float(np.asarray(res.results[k]["loss8"], np.float64).sum())
    loss = np.float32(total / NROWS)
    return loss, res


def kernel(z_i: np.ndarray, z_j: np.ndarray) -> np.ndarray:
    loss, _ = run_full(z_i, z_j, trace=False)
    return np.asarray(loss, dtype=np.float32)
